# revision 22
# baseline (speedup 1.0000x reference)
"""Bass/Trainium2 kernel for DocRE bilinear segment-reduce model.

Shapes (hardcoded): B=4, L=1024, H=768, NH=12, E=24, M=4, P=552, NL=97, BLK=64.
Sharding: 8 cores = (batch b = core//2) x (half of the 552 head-tail pairs).
One SPMD program; all per-core differences flow through input data
(gathered rows + one-hot matrices built on host from the integer indices).

v2 layout notes:
 - all GEMMs bf16 (fp32 matmuls cost 4 cyc/row on the PE)
 - WBT prefetched whole in a [128, 384*97] big-line layout; main-GEMM lhsT
   slices come straight out of SBUF (no per-chunk weight DMA)
 - phases 2+4 fused into a per-l-chunk (q) pipeline
 - phase 9 bilinear: zh spilled to DRAM; most chunks get their replicated-zh
   tile via broadcast DMA (off the PE) and a 2x-rate bf16 DVE multiply;
   the rest use PE one-hot replication with multiplies on Pool/Act+DVE
"""

import dataclasses
import numpy as np
import ml_dtypes

import concourse.bass as bass
import concourse.bacc as bacc
import concourse.tile as tile
from concourse import mybir
from concourse.bass_utils import run_bass_kernel_spmd

B, L, H, NH, E, M, P, NL, BLK = 4, 1024, 768, 12, 24, 4, 552, 97, 64
G = H // BLK            # 12 blocks
R = P // 2              # 276 rows per core
EM = E * M              # 96 gathered mentions
HL = NH * L             # 12288
K = H * BLK             # 49152 bilinear contraction
NCHK = K // 128         # 384 k-chunks
F32 = mybir.dt.float32
BF16 = mybir.dt.bfloat16

# phase-9 chunk split per g: c in [0, NDMA) replicated via DMA, the rest via PE
NDMA = 24
NPE = 32 - NDMA
DBLK = 8                # chunks per rep-block DMA (NDMA % DBLK == 0)
# engine for the NPE PE-replicated chunks' multiplies (cycled); GPSIMD cannot
# read PSUM, so "actpool"/"actdve" evacuate via the scalar engine first
PE_MUL = ["actdve", "actpool", "actdve", "actpool",
          "actdve", "actpool", "actdve", "actpool"]

_CACHE = {}


def _build_program(dbg=False):
    nc = bacc.Bacc("TRN2", target_bir_lowering=False, debug=False, num_devices=8)
    dp = nc.declare_dram_parameter
    if dbg:
        DBG = {nm: dp(nm, shp, BF16, isOutput=True) for nm, shp in (
            ("ZHOUT", [768, R]), ("ZTOUT", [768, R]), ("HTOUT", [1024, R]),
            ("RSOUT", [768, R]), ("ZTROUT", [128 * G, R]), ("REPOUT", [128 * G, R]),
            ("TSBOUT", [128, 3 * R]), ("W4OUT", [128, NH * R]))}
    EMG = dp("EMG", [EM, H], F32, isOutput=False)          # gathered mention embeddings
    SUME = dp("SUME", [EM, 128], BF16, isOutput=False)     # mask one-hot (logsumexp sum)
    W2H = dp("W2H", [EM, R], BF16, isOutput=False)         # mention->pair head weights
    W2T = dp("W2T", [EM, R], BF16, isOutput=False)         # mention->pair tail weights
    AMGQ = dp("AMGQ", [8 * EM, NH * 128], BF16, isOutput=False)  # attn gather, q-major
    OHH = dp("OHH", [128, R], BF16, isOutput=False)        # head-entity one-hot
    OHT = dp("OHT", [128, R], BF16, isOutput=False)        # tail-entity one-hot
    SEQ = dp("SEQ", [L, H], BF16, isOutput=False)          # sequence_output[b]
    WHT = dp("WHT", [2 * H, H], BF16, isOutput=False)      # Wh.T
    WTT = dp("WTT", [2 * H, H], BF16, isOutput=False)      # Wt.T
    WBTL = dp("WBTL", [128, NCHK * NL], BF16, isOutput=False)  # Wb.T chunk-major
    SELP = dp("SELP", [128, NPE * 128], BF16, isOutput=False)  # PE-rep one-hots
    BHS = dp("BHS", [128, 6], F32, isOutput=False)         # bh as [128,6] per o-chunk
    BTS = dp("BTS", [128, 6], F32, isOutput=False)
    BBS = dp("BBS", [NL, 1], F32, isOutput=False)
    OUT = dp("OUT", [NL, R], F32, isOutput=True)           # logits^T

    with tile.TileContext(nc) as tc:
        with (
            tc.tile_pool(name="persist", bufs=1) as pp,
            tc.tile_pool(name="amgs", bufs=3) as amgp,
            tc.tile_pool(name="w4s", bufs=2) as w4p,
            tc.tile_pool(name="tsbs", bufs=3) as tsbp,
            tc.tile_pool(name="wstream", bufs=3) as wp,
            tc.tile_pool(name="repblk", bufs=3) as repp,
            tc.tile_pool(name="blt", bufs=8) as bltp,
            tc.tile_pool(name="rsb", bufs=3) as rsbp,
            tc.tile_pool(name="dram", bufs=1, space="DRAM") as dramp,
        ):
            # ---- persistent small inputs
            def load(name, ap, shape, tag, dt=F32, eng=nc.sync):
                t = pp.tile(shape, dt, tag=tag, name=tag)
                eng.dma_start(t[:], ap)
                return t

            emg = load("EMG", EMG[:], [EM, H], "emg")
            sume = load("SUME", SUME[:], [EM, 128], "sume", BF16)
            w2h = load("W2H", W2H[:], [EM, R], "w2h", BF16)
            w2t = load("W2T", W2T[:], [EM, R], "w2t", BF16)
            ohh = load("OHH", OHH[:], [128, R], "ohh", BF16)
            oht = load("OHT", OHT[:], [128, R], "oht", BF16)
            bhs = load("BHS", BHS[:], [128, 6], "bhs")
            bts = load("BTS", BTS[:], [128, 6], "bts")
            bbs = load("BBS", BBS[:], [NL, 1], "bbs")
            selp = load("SELP", SELP[:], [128, NPE * 128], "selp", BF16)
            seqt = [load("SEQ", SEQ[q * 128:(q + 1) * 128, :], [128, H], f"seq{q}", BF16)
                    for q in range(8)]
            # full WBT prefetch, 12 slices on the scalar queue
            wbtl = pp.tile([128, NCHK * NL], BF16, tag="wbtl", name="wbtl")
            for s in range(12):
                nc.scalar.dma_start(
                    wbtl[:, s * 32 * NL:(s + 1) * 32 * NL],
                    WBTL[:, s * 32 * NL:(s + 1) * 32 * NL])
            ones = pp.tile([128, 128], F32, tag="ones", name="ones")
            nc.vector.memset(ones[:], 1.0)
            onesb = pp.tile([128, 1], BF16, tag="onesb", name="onesb")
            nc.vector.memset(onesb[:], 1.0)

            zhd = dramp.tile([768, R], BF16, tag="zhd", name="zhd")
            ztd = dramp.tile([768, R], BF16, tag="ztd", name="ztd")

            # ---- phase 1: entity embeddings = ln(sum_m mask * exp(m_emb))
            expt = pp.tile([EM, H], BF16, tag="expt", name="expt")
            nc.scalar.activation(expt[:], emg[:], mybir.ActivationFunctionType.Exp)
            eet = pp.tile([128, H], BF16, tag="eet", name="eet")
            with tc.tile_pool(name="ps1", bufs=2, space="PSUM") as ps1:
                for half in range(2):
                    pe = ps1.tile([128, 384], F32, tag="ee_ps", name="ee_ps")
                    nc.tensor.matmul(pe[:], sume[:], expt[:, half * 384:(half + 1) * 384],
                                     start=True, stop=True)
                    nc.scalar.activation(eet[:, half * 384:(half + 1) * 384], pe[:],
                                         mybir.ActivationFunctionType.Ln)

            # ---- phase 3: hs^T / ts^T gathers  [128d, R] x 6
            hst, tst = [], []
            with tc.tile_pool(name="ps3", bufs=4, space="PSUM") as ps3:
                for oc in range(6):
                    for si, (oh, dst_list, tag) in enumerate(
                            ((ohh, hst, "hs"), (oht, tst, "ts"))):
                        rg = ((oc * 2 + si) % 4) * 32
                        pg = ps3.tile([128, R], F32, tag="gat_ps", name="gat_ps")
                        nc.tensor.matmul(pg[:],
                                         eet[rg:rg + E, oc * 128:(oc + 1) * 128],
                                         oh[rg:rg + E, :],
                                         start=True, stop=True,
                                         tile_position=(rg, 0))
                        t = pp.tile([128, R], BF16, tag=f"{tag}{oc}", name=f"{tag}{oc}")
                        if (oc + si) % 2 == 0:
                            nc.scalar.copy(t[:], pg[:])
                        else:
                            nc.vector.tensor_copy(t[:], pg[:])
                        dst_list.append(t)

            # ---- phase 4 per l-chunk q: the mention->entity normalization is
            # folded into W2H/W2T on the host, so the per-(q,h) gathers
            # contract straight over the 96 gathered mentions of AMGQ.
            htacc = []
            with (
                tc.tile_pool(name="ps4h", bufs=2, space="PSUM") as ps4h,
                tc.tile_pool(name="ps4t", bufs=2, space="PSUM") as ps4t,
            ):
                for q in range(8):
                    amg_q = amgp.tile([EM, NH * 128], BF16, tag="amgq", name="amgq")
                    nc.sync.dma_start(amg_q[:], AMGQ[q * EM:(q + 1) * EM, :])

                    w4 = w4p.tile([128, NH, R], BF16, tag="w4", name="w4")
                    for hp in range(6):          # head pairs; 512-col slots keep
                        # each matmul output inside a single 2KB PSUM bank
                        h2 = ps4h.tile([128, 2, 512], F32, tag="h2_ps", name="h2_ps")
                        t2 = ps4t.tile([128, 2, 512], F32, tag="t2_ps", name="t2_ps")
                        for kk in range(2):
                            h = hp * 2 + kk
                            nc.tensor.matmul(h2[:, kk, 0:R],
                                             amg_q[:, h * 128:(h + 1) * 128],
                                             w2h[:], start=True, stop=True)
                            nc.tensor.matmul(t2[:, kk, 0:R],
                                             amg_q[:, h * 128:(h + 1) * 128],
                                             w2t[:], start=True, stop=True)
                        tsb = tsbp.tile([128, 2, R], BF16, tag="tsb", name="tsb")
                        nc.scalar.copy(tsb[:], t2[:, :, 0:R])
                        if hp < 5:
                            nc.vector.tensor_tensor(w4[:, hp * 2:hp * 2 + 2, :],
                                                    h2[:, :, 0:R], tsb[:],
                                                    mybir.AluOpType.mult)
                        else:
                            hsb = tsbp.tile([128, 2, R], BF16, tag="hsb", name="hsb")
                            nc.scalar.copy(hsb[:], h2[:, :, 0:R])
                            nc.vector.tensor_tensor(w4[:, hp * 2:hp * 2 + 2, :],
                                                    hsb[:], tsb[:],
                                                    mybir.AluOpType.mult)
                    if dbg and q == 0:
                        nc.sync.dma_start(
                            DBG["W4OUT"][:].rearrange("p (j r) -> p j r", j=NH), w4[:])
                    # tree-sum over the 12 heads
                    acc = pp.tile([128, R], BF16, tag=f"ht{q}", name=f"ht{q}")
                    nc.vector.tensor_add(w4[:, 0:6, :], w4[:, 0:6, :], w4[:, 6:12, :])
                    nc.vector.tensor_add(w4[:, 0:3, :], w4[:, 0:3, :], w4[:, 3:6, :])
                    nc.gpsimd.tensor_add(w4[:, 0, :], w4[:, 0, :], w4[:, 1, :])
                    nc.gpsimd.tensor_add(acc[:], w4[:, 0, :], w4[:, 2, :])
                    if dbg:
                        nc.sync.dma_start(DBG["HTOUT"][q * 128:(q + 1) * 128, :], acc[:])
                    htacc.append(acc)

            # ---- phase 5: 1/(sum_l ht + 1e-5), broadcast to 128 partitions
            invd = pp.tile([128, R], F32, tag="invd", name="invd")
            with tc.tile_pool(name="ps5", bufs=1, space="PSUM") as ps5:
                psum_s = ps5.tile([1, R], F32, tag="s_ps", name="s_ps")
                for q in range(8):
                    nc.tensor.matmul(psum_s[:], onesb[:], htacc[q][:],
                                     start=(q == 0), stop=(q == 7))
                invd1 = pp.tile([1, R], F32, tag="invd1", name="invd1")
                nc.vector.tensor_scalar_add(invd1[:], psum_s[:], 1e-5)
                nc.vector.reciprocal(invd1[:], invd1[:])
                pb = ps5.tile([128, R], F32, tag="invd_ps", name="invd_ps")
                nc.tensor.matmul(pb[:], ones[0:1, :], invd1[:], start=True, stop=True)
                nc.scalar.copy(invd[:], pb[:])

            # ---- phase 6: rs^T chunks (normalization folded into evac)
            rst = []
            with tc.tile_pool(name="ps6", bufs=2, space="PSUM") as ps6:
                for dc in range(6):
                    pr = ps6.tile([128, R], F32, tag="rs_ps", name="rs_ps")
                    for q in range(8):
                        nc.tensor.matmul(pr[:], seqt[q][:, dc * 128:(dc + 1) * 128],
                                         htacc[q][:], start=(q == 0), stop=(q == 7))
                    t = pp.tile([128, R], BF16, tag=f"rs{dc}", name=f"rs{dc}")
                    nc.vector.tensor_mul(t[:], pr[:], invd[:])
                    if dbg:
                        nc.sync.dma_start(DBG["RSOUT"][dc * 128:(dc + 1) * 128, :], t[:])
                    rst.append(t)

            # ---- phase 7: zh^T = tanh(Wh^T @ [hs; rs] + bh), same for zt;
            #      spill both to DRAM for the phase-9 broadcast DMAs
            zht, ztt = [], []
            for (wdram, inv, bias, out_list, tag, dspill) in (
                    (WHT, hst, bhs, zht, "zh", zhd), (WTT, tst, bts, ztt, "zt", ztd)):
                with tc.tile_pool(name=f"ps7{tag}", bufs=1, space="PSUM") as ps7:
                    pps = [ps7.tile([128, R], F32, tag=f"{tag}_ps{oc}",
                                    name=f"{tag}_ps{oc}") for oc in range(6)]
                    for k2 in range(12):
                        wt2 = wp.tile([128, H], BF16, tag="wproj", name="wproj")
                        nc.sync.dma_start(wt2[:], wdram[k2 * 128:(k2 + 1) * 128, :])
                        rhs = inv[k2] if k2 < 6 else rst[k2 - 6]
                        for oc in range(6):
                            nc.tensor.matmul(pps[oc][:],
                                             wt2[:, oc * 128:(oc + 1) * 128],
                                             rhs[:], start=(k2 == 0), stop=(k2 == 11))
                    for oc in range(6):
                        t = pp.tile([128, R], BF16, tag=f"{tag}{oc}", name=f"{tag}{oc}")
                        nc.scalar.activation(t[:], pps[oc][:],
                                             mybir.ActivationFunctionType.Tanh,
                                             bias=bias[:, oc:oc + 1])
                        nc.sync.dma_start(dspill[oc * 128:(oc + 1) * 128, :], t[:])
                        if dbg:
                            dbgt = DBG["ZHOUT"] if tag == "zh" else DBG["ZTOUT"]
                            nc.sync.dma_start(dbgt[oc * 128:(oc + 1) * 128, :], t[:])
                        out_list.append(t)

            # ---- phase 8: ztr[g] = [zt_g; zt_g] via broadcast DMA from DRAM
            ztr = []
            for g in range(G):
                t = pp.tile([128, R], BF16, tag=f"ztr{g}", name=f"ztr{g}")
                src = ztd[0:768, :]
                srcap = dataclasses.replace(
                    src, ap=[[0, 2], [R, 64], [1, R]], offset=src.offset + g * 64 * R)
                nc.scalar.dma_start(t[:], srcap)
                if dbg:
                    nc.sync.dma_start(DBG["ZTROUT"][g * 128:(g + 1) * 128, :], t[:])
                ztr.append(t)

            # ---- phase 9: bilinear logits^T = sum_k WbT[k,:]^T * bl^T[k,:]
            with (
                tc.tile_pool(name="ps9", bufs=1, space="PSUM") as ps9,
                tc.tile_pool(name="ps9r", bufs=3, space="PSUM") as ps9r,
            ):
                lt = ps9.tile([NL, R], F32, tag="lt_ps", name="lt_ps")
                ci = 0
                for g in range(G):
                    oc, gg = g // 2, g % 2
                    zsrc = zht[oc][gg * 64:(gg + 1) * 64, :]
                    # replicated-zh blocks for the DMA chunks of this g
                    blocks = []
                    for b0 in range(NDMA // DBLK):
                        repb = repp.tile([128, DBLK, R], BF16, tag="repb", name="repb")
                        base = (g * 64 + 2 * b0 * DBLK) * R
                        src = zhd[0:768, :]
                        for hh in range(2):
                            srcap = dataclasses.replace(
                                src, ap=[[0, 64], [2 * R, DBLK], [1, R]],
                                offset=src.offset + base + hh * R)
                            nc.sync.dma_start(repb[hh * 64:(hh + 1) * 64, :, :], srcap)
                        if dbg and b0 == 0:
                            nc.sync.dma_start(DBG["REPOUT"][g * 128:(g + 1) * 128, :],
                                              repb[:, 0, :])
                        blocks.append(repb)
                    ztb4 = dataclasses.replace(
                        ztr[g][:], ap=[ztr[g][:].ap[0], [0, 4], [1, R]])
                    for c in range(32):
                        if c < NDMA:
                            if c % 4 == 0:
                                blk = blocks[c // DBLK]
                                j0 = c % DBLK
                                blt4 = bltp.tile([128, 4, R], BF16, tag="blt4",
                                                 name="blt4")
                                nc.vector.tensor_tensor(
                                    blt4[:], blk[:, j0:j0 + 4, :], ztb4,
                                    mybir.AluOpType.mult)
                            blt = blt4[:, c % 4, :]
                        else:
                            v = c - NDMA
                            rep = ps9r.tile([128, R], F32, tag="rep_ps", name="rep_ps")
                            nc.tensor.matmul(rep[:],
                                             selp[gg * 64:(gg + 1) * 64,
                                                  v * 128:(v + 1) * 128],
                                             zsrc, start=True, stop=True,
                                             tile_position=(gg * 64, 0))
                            bltt = bltp.tile([128, R], BF16, tag="blt1", name="blt1")
                            eng = PE_MUL[v]
                            if eng == "dve":
                                nc.vector.tensor_tensor(bltt[:], rep[:], ztr[g][:],
                                                        mybir.AluOpType.mult)
                            else:
                                rsb = rsbp.tile([128, R], BF16, tag="rsb", name="rsb")
                                nc.scalar.copy(rsb[:], rep[:])
                                if eng == "actpool":
                                    nc.gpsimd.tensor_tensor(bltt[:], rsb[:], ztr[g][:],
                                                            mybir.AluOpType.mult)
                                else:
                                    nc.vector.tensor_tensor(bltt[:], rsb[:], ztr[g][:],
                                                            mybir.AluOpType.mult)
                            blt = bltt[:]
                        nc.tensor.matmul(lt[:], wbtl[:, ci * NL:(ci + 1) * NL], blt,
                                         start=(ci == 0), stop=(ci == NCHK - 1))
                        ci += 1

                lout = pp.tile([NL, R], F32, tag="lout", name="lout")
                nc.vector.tensor_scalar_add(lout[:], lt[:], bbs[:, 0:1])
                nc.sync.dma_start(OUT[:], lout[:])

    nc.finalize()
    return nc


def _prep_core_inputs(c, sequence_output, attention, mention_mask, Wh, bh, Wt, bt,
                      Wb, bb, mention_idx, hts):
    bf16 = ml_dtypes.bfloat16
    b, half = c // 2, c % 2
    seq_b = np.ascontiguousarray(sequence_output[b])              # [L, H]
    idx = mention_idx[b].astype(np.int64).reshape(EM)             # [96]
    mask = mention_mask[b].astype(np.float32)                     # [E, M]
    denom = mask.sum(-1)                                          # [E]

    emg = np.ascontiguousarray(seq_b[idx])                        # [96, H]
    # attention gather, q-major: [8*EM, NH*128]
    gm = attention[b][:, idx, :].transpose(1, 0, 2)               # [EM, NH, L]
    amgq = np.ascontiguousarray(
        gm.reshape(EM, NH, 8, 128).transpose(2, 0, 1, 3).reshape(8 * EM, NH * 128))

    sume = np.zeros((EM, 128), np.float32)
    base_w = np.zeros((EM, E), np.float32)
    s = 1.0 / np.sqrt(np.float32(NH))
    for e in range(E):
        for m in range(M):
            for rg in range(4):
                sume[e * M + m, rg * 32 + e] = mask[e, m]
            base_w[e * M + m, e] = mask[e, m] / denom[e] * s
    # unused gap partitions (rows 24-31 of each group): keep their exp-sums
    # positive so the Ln over the full [128, .] tile stays finite
    for rg in range(4):
        sume[0, rg * 32 + E:rg * 32 + 32] = 1.0

    hts_c = hts[b, half * R:(half + 1) * R].astype(np.int64)      # [R, 2]
    ohh = np.zeros((128, R), np.float32)
    oht = np.zeros((128, R), np.float32)
    for rg in range(4):
        ohh[rg * 32 + hts_c[:, 0], np.arange(R)] = 1.0
        oht[rg * 32 + hts_c[:, 1], np.arange(R)] = 1.0
    w2h = np.ascontiguousarray(base_w[:, hts_c[:, 0]])            # [EM, R]
    w2t = np.ascontiguousarray(base_w[:, hts_c[:, 1]])            # [EM, R]

    # WBT chunk-major big lines: WBTL[p, ci*NL+n] = Wb[n, k(ci, p)]
    wbt = Wb.T.astype(np.float32)                                 # [K, NL]
    ci_arr = np.arange(NCHK)[:, None]                             # [384, 1]
    p_arr = np.arange(128)[None, :]                               # [1, 128]
    kmat = ((ci_arr // 32) * 4096 + (2 * (ci_arr % 32) + p_arr // 64) * 64
            + (p_arr % 64))                                       # [384, 128]
    wbtl = np.ascontiguousarray(
        wbt[kmat].transpose(1, 0, 2).reshape(128, NCHK * NL))

    # PE-rep one-hots: selp[f, v*128+p] = 1 iff f%64 == 2*(NDMA+v) + p//64
    selp = np.zeros((128, NPE * 128), np.float32)
    for v in range(NPE):
        cc = NDMA + v
        for p in range(128):
            f = 2 * cc + p // 64
            selp[f, v * 128 + p] = 1.0
            selp[64 + f, v * 128 + p] = 1.0

    return {
        "EMG": emg.astype(np.float32), "SUME": sume.astype(bf16),
        "W2H": w2h.astype(bf16), "W2T": w2t.astype(bf16),
        "AMGQ": amgq.astype(bf16),
        "OHH": ohh.astype(bf16), "OHT": oht.astype(bf16),
        "SEQ": seq_b.astype(bf16),
        "WHT": np.ascontiguousarray(Wh.T).astype(bf16),
        "WTT": np.ascontiguousarray(Wt.T).astype(bf16),
        "WBTL": wbtl.astype(bf16), "SELP": selp.astype(bf16),
        "BHS": np.ascontiguousarray(bh.reshape(6, 128).T).astype(np.float32),
        "BTS": np.ascontiguousarray(bt.reshape(6, 128).T).astype(np.float32),
        "BBS": bb.reshape(NL, 1).astype(np.float32),
    }


def kernel(sequence_output, attention, mention_mask, Wh, bh, Wt, bt, Wb, bb,
           mention_idx, hts):
    if "nc" not in _CACHE:
        _CACHE["nc"] = _build_program()
    nc = _CACHE["nc"]

    args = (np.asarray(sequence_output, np.float32), np.asarray(attention, np.float32),
            np.asarray(mention_mask, np.float32), np.asarray(Wh, np.float32),
            np.asarray(bh, np.float32), np.asarray(Wt, np.float32),
            np.asarray(bt, np.float32), np.asarray(Wb, np.float32),
            np.asarray(bb, np.float32), np.asarray(mention_idx),
            np.asarray(hts))
    in_maps = [_prep_core_inputs(c, *args) for c in range(8)]
    try:
        res = run_bass_kernel_spmd(nc, in_maps, list(range(8))).results
    except Exception:
        # transient NRT_EXEC_UNIT_UNRECOVERABLE has been observed on the
        # first execution of a freshly loaded NEFF; retry once
        res = run_bass_kernel_spmd(nc, in_maps, list(range(8))).results

    out = np.empty((B, P, NL), np.float32)
    for c in range(8):
        b, half = c // 2, c % 2
        out[b, half * R:(half + 1) * R, :] = np.asarray(res[c]["OUT"]).T
    return out
